# revision 4
# baseline (speedup 1.0000x reference)
"""BasicGCN (2-layer GCN, 100K nodes / 3.2M edges) on 8 Trainium2 NeuronCores.

v2 strategy (node/dst sharding, graph-parallel, commuted transforms):
  Since segment-sum commutes with the dense transforms,
      out1 = relu(dinv_d * (segsum_e dinv_s x[s]) @ W1 + b1)
      out2 = logsoftmax(dinv_d * segsum_e (dinv_s * (out1 @ W2)[s]) + b2)
  no dense pre-pass over all nodes is needed: layer 1 gathers raw
  dinv-scaled x rows (bf16, 512B each), and the W1/W2 matmuls run after
  aggregation on each core's 12544 dst rows only.  Layer 2 gathers the
  64-wide zw = dinv*(relu(...)@W2) rows (bf16 + 64-col zero pad = 256B).

  - Pad nodes to NPAD = 100352 = 8 * 12544; core c owns dst rows
    [c*12544, (c+1)*12544).
  - Host preprocessing (index-space + dinv row scaling): degrees/dinv,
    xs = dinv*x as a bf16 [NPAD, 256] gather table, per-core edge streams
    bucketed by (superquad of SQ dst tiles, src-group, dst-tile); self
    loops are NOT in the stream (handled as an identity matmul over each
    dst tile's own contiguous rows).  Per-(tile,group) slot quotas are
    equalized across cores so one SPMD program serves all 8 cores.
  - Device per core, layer 1: dma_gather xs rows in <=1024-row calls
    spanning a (superquad, group) run; one-hot S blocks built on DVE
    (S[e,d] = dst_local[e]==d); segment-sum via PE bf16 matmuls into
    per-tile f32 PSUM accumulators (identity matmul adds the self loop);
    epilogue per tile (PE/Act only):
      o1 = dinv_d*aggX ; o1T (PE transpose) ; zT = W1.T@o1T ;
      z2T = relu(zT + b1) (Act, bias per-partition in transposed layout) ;
      zw = dinv_d*(z2T.T@W2)  -> bf16 row [64 data + 64 zeros] -> zw_own.
  - AllGather zw shards -> zw_full [NPAD, 128] bf16 (Shared DRAM).
  - Layer 2: same gather/S/matmul schedule with 256B rows from zw_full,
    epilogue log_softmax (f32) -> out shard [12544, 64].
  - Host: concatenate 8 shards, trim to [100000, 64].

Gather tables are split into 4 row-groups of NPAD/4 = 25088 rows so the
int16 gather indices stay in range; each dma_gather call is capped at
QMAX=1024 indices (the q7 firmware rejects more; verified empirically,
independent of dynamic_dma_scratch_size) and spans a (superquad, group)
run to keep calls full (the 994ns SWDGE fixed cost per call is the main
Pool-engine expense).
"""

import numpy as np

import concourse.bacc as bacc
import concourse.bass as bass
import concourse.mybir as mybir
import concourse.tile as tile
from concourse.bass_utils import run_bass_kernel_spmd

F32 = mybir.dt.float32
BF16 = mybir.dt.bfloat16
I16 = mybir.dt.int16
NP_BF16 = mybir.dt.np(BF16)
AF = mybir.ActivationFunctionType
ALU = mybir.AluOpType

N_CORES = 8
PAD_DSTLOC = 1000.0  # sentinel dst-local for padding slots -> zero S column
QMAX = 1024  # max num_idxs per dma_gather call (hard q7 firmware limit)
SQ = 6       # dst tiles per superquad: SQ agg psum banks + 2 epi banks = 8


def make_cfg(n_nodes=100000, d_in=256, d_hid=256, d_out=64, shard_tiles=98,
             n_groups=4):
    shard = shard_tiles * 128
    npad = N_CORES * shard
    assert npad % n_groups == 0
    gr = npad // n_groups
    assert gr <= 32768
    assert n_nodes <= npad
    return dict(N=n_nodes, NPAD=npad, SHARD=shard, NT=shard_tiles,
                NG=n_groups, GR=gr, D_IN=d_in, D_HID=d_hid, D_OUT=d_out)


FULL_CFG = make_cfg()


def _build_schedule(quota, nt, ng):
    """Gather-call schedule over (superquad, group) runs.

    Returns (calls, blk_tile, call_off_flat, slot_total):
      calls: list of (g, slot_off, q) in stream order, q <= QMAX, all %128==0
      blk_tile: tile id per 128-slot block, in stream order
      call_off_flat[t*ng+g]: slot offset of the (t,g) section
    """
    call_off_flat = np.zeros(nt * ng, np.int64)
    blk_tile = []
    calls = []
    off = 0
    for sq in range(0, nt, SQ):
        tiles = range(sq, min(sq + SQ, nt))
        for g in range(ng):
            total = 0
            for t in tiles:
                q = int(quota[t, g])
                call_off_flat[t * ng + g] = off + total
                blk_tile.extend([t] * (q // 128))
                total += q
            if total == 0:
                continue
            nblk = total // 128
            nch = (total + QMAX - 1) // QMAX
            base, rem = divmod(nblk, nch)
            o = off
            for i in range(nch):
                q = (base + (1 if i < rem else 0)) * 128
                calls.append((g, o, q))
                o += q
            off += total
    return calls, blk_tile, call_off_flat, off


# --------------------------------------------------------------------------
# Host preprocessing
# --------------------------------------------------------------------------

def preprocess(x, edge_index, W1, b1, W2, b2, cfg):
    N, NPAD, SHARD, NT, NG, GR = (cfg["N"], cfg["NPAD"], cfg["SHARD"],
                                  cfg["NT"], cfg["NG"], cfg["GR"])
    D_IN, D_HID, D_OUT = cfg["D_IN"], cfg["D_HID"], cfg["D_OUT"]

    x = np.asarray(x, np.float32)
    edge_index = np.asarray(edge_index)
    src = edge_index[0].astype(np.int64)
    dst = edge_index[1].astype(np.int64)

    deg = np.bincount(dst, minlength=N).astype(np.float32) + 1.0
    dinv = 1.0 / np.sqrt(deg)
    dinv_pad = np.zeros(NPAD, np.float32)
    dinv_pad[:N] = dinv

    E = src.shape[0]

    c_of = dst // SHARD
    t_of = (dst % SHARD) // 128
    d_of = (dst % 128).astype(np.float32)
    g_of = src // GR
    srcg = (src % GR).astype(np.int16)

    key = (c_of * NT + t_of) * NG + g_of
    order = np.argsort(key, kind="stable")
    counts = np.bincount(key, minlength=N_CORES * NT * NG)
    quota = counts.reshape(N_CORES, NT, NG).max(axis=0)
    quota = ((quota + 127) // 128) * 128  # round up to whole 128-slot blocks

    calls, blk_tile, call_off_flat, slot_total = _build_schedule(
        quota, NT, NG)

    # slot position of each edge inside its core's stream
    csum = np.zeros(N_CORES * NT * NG + 1, np.int64)
    np.cumsum(counts, out=csum[1:])
    sorted_key = key[order]
    rank = np.arange(E, dtype=np.int64) - csum[sorted_key]
    tg = t_of[order] * NG + g_of[order]
    slot = call_off_flat[tg] + rank
    core = c_of[order]

    idx_arr = np.zeros((N_CORES, slot_total), np.int16)  # pad -> row 0
    dl_arr = np.full((N_CORES, slot_total), PAD_DSTLOC, np.float32)
    idx_arr[core, slot] = srcg[order]
    dl_arr[core, slot] = d_of[order]

    # global wrapping (consistent for any 128-aligned call offset):
    # idx wrapped [16, slots/16] replicated to 128 parts; dl wrapped
    # [128, slots/128]
    idxcols = slot_total // 16
    nb = slot_total // 128
    idx_sb = idx_arr.reshape(N_CORES, idxcols, 16).transpose(0, 2, 1)
    idx_sb = np.ascontiguousarray(np.tile(idx_sb, (1, 8, 1)))
    dl_sb = np.ascontiguousarray(
        dl_arr.reshape(N_CORES, nb, 128).transpose(0, 2, 1))

    # dense gather table: xs = dinv * x, padded, bf16, row-major
    xs = np.zeros((NPAD, D_IN), NP_BF16)
    xs[:N] = (dinv[:, None] * x).astype(NP_BF16)

    ntile = NPAD // 128
    dinv_nodes = np.ascontiguousarray(
        dinv_pad.reshape(ntile, 128).T)  # [128, ntile]
    dinv_dst = np.stack([dinv_nodes[:, c * NT:(c + 1) * NT]
                         for c in range(N_CORES)])  # [8, 128, NT]

    iota = np.tile(np.arange(128), (128, 1)).astype(NP_BF16)
    ident = np.eye(128, dtype=NP_BF16)
    identf = np.eye(128, dtype=np.float32)
    # b1 per-partition column (features on partitions in transposed layout)
    b1col = np.ascontiguousarray(
        np.asarray(b1, np.float32).reshape(D_HID // 128, 128).T)  # [128, KH]
    b2bc = np.ascontiguousarray(
        np.broadcast_to(np.asarray(b2, np.float32), (128, D_OUT)))

    common = dict(xs=xs, W1=np.asarray(W1, NP_BF16),
                  W2=np.asarray(W2, NP_BF16), b1col=b1col, b2bc=b2bc,
                  iota=iota, ident=ident, identf=identf)
    in_maps = []
    for c in range(N_CORES):
        m = dict(common)
        m["xs_own"] = np.ascontiguousarray(xs[c * SHARD:(c + 1) * SHARD])
        m["dinv_dst"] = np.ascontiguousarray(dinv_dst[c])
        m["idx_sb"] = np.ascontiguousarray(idx_sb[c])
        m["dstloc"] = np.ascontiguousarray(dl_sb[c])
        in_maps.append(m)

    meta = dict(quota=quota, idxcols=idxcols, nb=nb, calls=calls,
                blk_tile=blk_tile)
    return in_maps, meta


# --------------------------------------------------------------------------
# Device program
# --------------------------------------------------------------------------

def build_program(cfg, meta, with_collective=True, phases=(2, 3)):
    NPAD, NT, NG, GR = cfg["NPAD"], cfg["NT"], cfg["NG"], cfg["GR"]
    D_IN, D_HID, D_OUT = cfg["D_IN"], cfg["D_HID"], cfg["D_OUT"]
    SHARD = cfg["SHARD"]
    idxcols, nb = meta["idxcols"], meta["nb"]
    calls, blk_tile = meta["calls"], meta["blk_tile"]
    KI = D_IN // 128   # k-chunks of x features
    KH = D_HID // 128  # k-chunks of hidden features
    CMAX = QMAX // 128
    D_L2 = 2 * D_OUT  # layer-2 table row: 64 bf16 data + 64 bf16 zeros

    # first/last block of each tile (accumulation start/stop flags)
    first_blk = {}
    last_blk = {}
    for i, t in enumerate(blk_tile):
        first_blk.setdefault(t, i)
        last_blk[t] = i

    nc = bacc.Bacc("TRN2", target_bir_lowering=False, debug=False,
                   num_devices=N_CORES)

    xs_d = nc.dram_tensor("xs", [NPAD, D_IN], BF16, kind="ExternalInput")
    xso_d = nc.dram_tensor("xs_own", [SHARD, D_IN], BF16,
                           kind="ExternalInput")
    W1_d = nc.dram_tensor("W1", [D_IN, D_HID], BF16, kind="ExternalInput")
    W2_d = nc.dram_tensor("W2", [D_HID, D_OUT], BF16, kind="ExternalInput")
    b1_d = nc.dram_tensor("b1col", [128, KH], F32, kind="ExternalInput")
    b2_d = nc.dram_tensor("b2bc", [128, D_OUT], F32, kind="ExternalInput")
    iota_d = nc.dram_tensor("iota", [128, 128], BF16, kind="ExternalInput")
    ident_d = nc.dram_tensor("ident", [128, 128], BF16, kind="ExternalInput")
    identf_d = nc.dram_tensor("identf", [128, 128], F32, kind="ExternalInput")
    dinvd_d = nc.dram_tensor("dinv_dst", [128, NT], F32, kind="ExternalInput")
    idx_d = nc.dram_tensor("idx_sb", [128, idxcols], I16, kind="ExternalInput")
    dl_d = nc.dram_tensor("dstloc", [128, nb], F32, kind="ExternalInput")
    out_d = nc.dram_tensor("out", [SHARD, D_OUT], F32, kind="ExternalOutput")

    with tile.TileContext(nc) as tc:
        with (
            tc.tile_pool(name="const", bufs=1) as const,
            tc.tile_pool(name="dram", bufs=1, space="DRAM") as dram,
        ):
            zw_own = dram.tile([SHARD, D_L2], BF16)
            zw_full = dram.tile([NPAD, D_L2], BF16, addr_space="Shared")

            w1_sb = const.tile([128, KI, D_HID], BF16)
            for k in range(KI):
                nc.sync.dma_start(out=w1_sb[:, k, :],
                                  in_=W1_d.ap()[k * 128:(k + 1) * 128, :])
            w2_sb = const.tile([128, KH, D_OUT], BF16)
            for k in range(KH):
                nc.sync.dma_start(out=w2_sb[:, k, :],
                                  in_=W2_d.ap()[k * 128:(k + 1) * 128, :])
            iota_sb = const.tile([128, 128], BF16)
            nc.sync.dma_start(out=iota_sb[:], in_=iota_d.ap())
            ident_sb = const.tile([128, 128], BF16)
            nc.sync.dma_start(out=ident_sb[:], in_=ident_d.ap())
            identf_sb = const.tile([128, 128], F32)
            nc.sync.dma_start(out=identf_sb[:], in_=identf_d.ap())
            b1_sb = const.tile([128, KH], F32)
            nc.sync.dma_start(out=b1_sb[:], in_=b1_d.ap())
            b2_sb = const.tile([128, D_OUT], F32)
            nc.sync.dma_start(out=b2_sb[:], in_=b2_d.ap())
            dinvd_sb = const.tile([128, NT], F32)
            nc.sync.dma_start(out=dinvd_sb[:], in_=dinvd_d.ap())
            idx_sb = const.tile([128, idxcols], I16)
            nc.sync.dma_start(out=idx_sb[:], in_=idx_d.ap())
            dl_sb = const.tile([128, nb], F32)
            nc.sync.dma_start(out=dl_sb[:], in_=dl_d.ap())

            zw_own_r = zw_own.rearrange("(t p) f -> t p f", p=128)
            xso_r = xso_d.ap().rearrange("(t p) f -> t p f", p=128)

            def agg_phase(table, self_rows, elem, rhs_w, epilogue,
                          mtag, stag, ptag):
                """Gather + one-hot-S + matmul accumulation over the
                precomputed superquad-spanning call schedule. self_rows(t)
                is a DRAM [128, elem] AP with the tile's own rows; their
                first rhs_w cols are added via identity matmul (self loop,
                scheduled at the tile's LAST block so the row DMA, issued
                at the first block, has a long prefetch window)."""
                blk = 0
                psums = {}
                sls = {}
                with (
                    tc.tile_pool(name=mtag, bufs=8) as mpool,
                    tc.tile_pool(name=stag, bufs=8) as spool,
                    tc.tile_pool(name=mtag + "sl", bufs=SQ + 2) as slpool,
                    tc.tile_pool(name=ptag, bufs=SQ, space="PSUM") as apsum,
                    tc.tile_pool(name=ptag + "ep", bufs=3) as eppool,
                    tc.tile_pool(name=ptag + "ep2", bufs=2,
                                 space="PSUM") as eppsum,
                ):
                    for g, o, q in calls:
                        ncols = q // 128
                        mt = mpool.tile([128, CMAX, elem], BF16, tag="m")
                        nc.gpsimd.dma_gather(
                            mt[:, :ncols, :],
                            table(g),
                            idx_sb[:, o // 16:(o + q) // 16],
                            q, q, elem)
                        for j in range(ncols):
                            t = blk_tile[blk]
                            if blk == first_blk[t]:
                                psums[t] = apsum.tile(
                                    [128, rhs_w], F32, tag="agg",
                                    name="aggps")
                                sl = slpool.tile([128, elem], BF16, tag="sl")
                                nc.sync.dma_start(out=sl[:], in_=self_rows(t))
                                sls[t] = sl
                            st = spool.tile([128, 128], BF16, tag="s",
                                            name="stile")
                            nc.vector.tensor_scalar(
                                st[:], iota_sb[:], dl_sb[:, blk:blk + 1],
                                None, ALU.is_equal)
                            nc.tensor.matmul(
                                psums[t][:], st[:], mt[:, j, :rhs_w],
                                start=(blk == first_blk[t]), stop=False)
                            if blk == last_blk[t]:
                                nc.tensor.matmul(
                                    psums[t][:], ident_sb[:],
                                    sls.pop(t)[:, :rhs_w],
                                    start=False, stop=True)
                                epilogue(t, psums.pop(t), eppool, eppsum)
                            blk += 1

            # ------------- layer 1 epilogue: zw = f(aggX) per tile ---------
            def epi1(t, ps, eppool, eppsum):
                # o1 = dinv_d * aggX  (Act copy, per-partition scale)
                o1 = eppool.tile([128, D_IN], F32, tag="o1")
                nc.scalar.activation(o1[:], ps[:], AF.Copy,
                                     scale=dinvd_sb[:, t:t + 1])
                # transpose o1 -> o1T (cast to bf16 via Act copy from psum)
                o1T = eppool.tile([128, KI, 128], BF16, tag="o1T")
                for k in range(KI):
                    tp = eppsum.tile([128, 128], F32, tag="ep")
                    nc.tensor.transpose(tp[:], o1[:, k * 128:(k + 1) * 128],
                                        identf_sb[:])
                    nc.scalar.activation(o1T[:, k, :], tp[:], AF.Copy)
                # zT_k = sum_j W1[j,k].T @ o1T_j ; z2T = relu(zT + b1col)
                z2T = eppool.tile([128, KH, 128], BF16, tag="z2T")
                for k in range(KH):
                    zps = eppsum.tile([128, 128], F32, tag="ep")
                    for j in range(KI):
                        nc.tensor.matmul(
                            zps[:], w1_sb[:, j, k * 128:(k + 1) * 128],
                            o1T[:, j, :],
                            start=(j == 0), stop=(j == KI - 1))
                    nc.scalar.activation(z2T[:, k, :], zps[:], AF.Relu,
                                         bias=b1_sb[:, k:k + 1])
                # zw = dinv_d * (z2.T @ W2) : lhsT = z2T chunks
                zwps = eppsum.tile([128, 128], F32, tag="ep")
                for k in range(KH):
                    nc.tensor.matmul(zwps[:, :D_OUT], z2T[:, k, :],
                                     w2_sb[:, k, :],
                                     start=(k == 0), stop=(k == KH - 1))
                zwsb = eppool.tile([128, D_L2], BF16, tag="zwsb")
                nc.vector.memset(zwsb[:, D_OUT:], 0.0)
                nc.scalar.activation(zwsb[:, :D_OUT], zwps[:, :D_OUT],
                                     AF.Copy, scale=dinvd_sb[:, t:t + 1])
                nc.sync.dma_start(out=zw_own_r[t], in_=zwsb[:])

            if 2 in phases:
                agg_phase(lambda g: xs_d.ap()[g * GR:(g + 1) * GR, :],
                          lambda t: xso_r[t], D_IN, D_IN, epi1,
                          "m1", "s1", "ag1")

            # ---------------- AllGather zw shards -------------------------
            if with_collective and 2 in phases:
                nc.gpsimd.collective_compute(
                    "AllGather", ALU.bypass,
                    replica_groups=[list(range(N_CORES))],
                    ins=[zw_own.opt()], outs=[zw_full.opt()])

            # ------------- layer 2 epilogue: log_softmax ------------------
            out_r = out_d.ap().rearrange("(t p) f -> t p f", p=128)

            def epi2(t, ps, eppool, eppsum):
                t0 = eppool.tile([128, D_OUT], F32, tag="t0")
                nc.vector.tensor_scalar(t0[:], ps[:], dinvd_sb[:, t:t + 1],
                                        None, ALU.mult)
                nc.vector.tensor_tensor(t0[:], t0[:], b2_sb[:], ALU.add)
                nm = eppool.tile([128, 1], F32, tag="nm")
                nc.vector.tensor_reduce(nm[:], t0[:], mybir.AxisListType.X,
                                        ALU.max, negate=True)
                et = eppool.tile([128, D_OUT], F32, tag="et")
                se = eppool.tile([128, 1], F32, tag="se")
                nc.scalar.activation(et[:], t0[:], AF.Exp, bias=nm[:],
                                     accum_out=se[:])
                ls = eppool.tile([128, 1], F32, tag="ls")
                nc.scalar.activation(ls[:], se[:], AF.Ln)
                ot = eppool.tile([128, D_OUT], F32, tag="ot")
                nc.vector.tensor_scalar(ot[:], t0[:], nm[:], ls[:],
                                        ALU.add, ALU.subtract)
                nc.sync.dma_start(out=out_r[t], in_=ot[:])

            if 3 in phases:
                agg_phase(lambda g: zw_full[g * GR:(g + 1) * GR, :],
                          lambda t: zw_own_r[t], D_L2, D_OUT, epi2,
                          "m2", "s2", "ag2")

    nc.compile()
    return nc


# --------------------------------------------------------------------------
# Entry point
# --------------------------------------------------------------------------

def kernel(x, edge_index, W1, b1, W2, b2):
    cfg = FULL_CFG
    in_maps, meta = preprocess(x, edge_index, W1, b1, W2, b2, cfg)
    nc = build_program(cfg, meta)
    res = run_bass_kernel_spmd(nc, in_maps, core_ids=list(range(N_CORES)))
    shards = [res.results[c]["out"] for c in range(N_CORES)]
    full = np.concatenate(shards, axis=0)
    return full[:cfg["N"]].astype(np.float32)
